# revision 2
# baseline (speedup 1.0000x reference)
"""Trainium2 Bass kernel for dual-attention (DisKT-style) nn module.

Math per (batch, head) with S=1024, dk=64:
    sT       = (k_h @ q_h^T) / sqrt(dk)                     # [k, q] transposed layout!
    E1T      = exp(sT) * causal(k < q)                      # first-softmax numerators
    r1[q]    = sum_k E1T[k, q]                              # first-softmax denominator
    p1m      = (E1T / r1) * (1 - cm)[k]                     # counter-masked probs
    E2T      = exp(p1m)   (dead entries exp(0)=1)           # second-softmax numerators
    out      = (E2T^T @ [v1|v2|ones]) ;  out /= r2 (=ones col);  out[q=0] = 0

Everything on-chip runs in "transposed" [k, q] layout so that:
  - the counter-attention mask is a per-partition scalar (activation scale),
  - the PV matmuls need no transpose at all (E2T chunks are the stationary
    operand directly),
  - the final 1/r2 normalization is a per-partition tensor_scalar at PSUM
    eviction time.
Causal structure is exploited: for k-chunk c only q > 128c is computed
("live" region); the causally-dead region contributes exp(0)=1 to the second
softmax, which is accounted for with a host-precomputed rank-1 fixup (vsuf).

Sharding: data-parallel over batch, B=16 -> 2 per core on 8 cores.
"""

import numpy as np

import concourse.bass as bass
import concourse.mybir as mybir
import concourse.tile as tile
from concourse import bacc
from concourse.bass_utils import run_bass_kernel_spmd

B, S, D, H = 16, 1024, 512, 8
DK = D // H           # 64
NCORES = 8
BLOC = B // NCORES    # 2 batches per core
NCH = S // 128        # 8 k-chunks of 128
VW = 132              # padded width of [v1|v2|ones] (129 used)
F32 = mybir.dt.float32

# knobs that test.py can flip
TRACE = False
LAST_RESULTS = None


def build_nc(debug=False):
    nc = bacc.Bacc("TRN2", target_bir_lowering=False, debug=debug)
    AF = mybir.ActivationFunctionType
    ALU = mybir.AluOpType

    qt_d = nc.dram_tensor("qt", [BLOC, H, DK, S], F32, kind="ExternalInput")
    kt_d = nc.dram_tensor("kt", [BLOC, H, DK, S], F32, kind="ExternalInput")
    vcat_d = nc.dram_tensor("vcat", [BLOC, H, NCH, 128, VW], F32, kind="ExternalInput")
    vsuf_d = nc.dram_tensor("vsuf", [BLOC, H, NCH, VW], F32, kind="ExternalInput")
    cms_d = nc.dram_tensor("cms", [BLOC, NCH, 128], F32, kind="ExternalInput")
    dmask_d = nc.dram_tensor("dmask", [128, 128], F32, kind="ExternalInput")
    ones_d = nc.dram_tensor("onesd", [128, 128], F32, kind="ExternalInput")
    out1_d = nc.dram_tensor("out1", [BLOC, S, D], F32, kind="ExternalOutput")
    out2_d = nc.dram_tensor("out2", [BLOC, S, D], F32, kind="ExternalOutput")

    with tile.TileContext(nc) as tc:
        with (
            tc.tile_pool(name="consts", bufs=1) as consts,
            tc.tile_pool(name="qk", bufs=2) as qkp,
            tc.tile_pool(name="vc", bufs=2) as vcp,
            tc.tile_pool(name="e1", bufs=1) as e1p,
            tc.tile_pool(name="e2", bufs=1) as e2p,
            tc.tile_pool(name="tmp", bufs=2) as tmpp,
            tc.tile_pool(name="rc", bufs=2) as rcp,
            tc.tile_pool(name="small", bufs=4) as smallp,
            tc.tile_pool(name="outs", bufs=4) as outp,
            tc.tile_pool(name="sc_ps", bufs=2, space="PSUM") as sc_psp,
            tc.tile_pool(name="r1_ps", bufs=1, space="PSUM") as r1_psp,
            tc.tile_pool(name="pv_ps", bufs=2, space="PSUM") as pv_psp,
        ):
            cm_sb = consts.tile([128, BLOC * NCH], F32)
            nc.sync.dma_start(out=cm_sb, in_=cms_d[:].rearrange("b c p -> p (b c)"))
            dm_sb = consts.tile([128, 128], F32)
            nc.sync.dma_start(out=dm_sb, in_=dmask_d[:, :])
            ones_sb = consts.tile([128, 128], F32)
            nc.sync.dma_start(out=ones_sb, in_=ones_d[:, :])

            for bi in range(BLOC):
                for h in range(H):
                    qt_sb = qkp.tile([DK, S], F32, tag="qt")
                    kt_sb = qkp.tile([DK, S], F32, tag="kt")
                    nc.sync.dma_start(out=qt_sb, in_=qt_d[bi, h])
                    nc.sync.dma_start(out=kt_sb, in_=kt_d[bi, h])
                    vc_sb = vcp.tile([128, NCH, VW], F32, tag="vc")
                    nc.sync.dma_start(
                        out=vc_sb, in_=vcat_d[bi, h].rearrange("c p j -> p c j")
                    )
                    vs_sb = vcp.tile([1, NCH, VW], F32, tag="vs")
                    nc.sync.dma_start(out=vs_sb, in_=vsuf_d[bi, h][None, :, :])

                    # ---- phase A: scoresT -> exp -> causal-diag mask -> r1 ----
                    r1ps = r1_psp.tile([128, S], F32, tag="r1")
                    e1 = e1p.tile([128, NCH, S], F32, tag="e1")
                    for c in range(NCH):
                        q0 = 128 * c
                        lw = S - q0
                        sps = sc_psp.tile([128, S], F32, tag="sc")
                        for n0 in range(0, lw, 512):
                            w = min(512, lw - n0)
                            nc.tensor.matmul(
                                sps[:, n0 : n0 + w],
                                lhsT=kt_sb[:, q0 : q0 + 128],
                                rhs=qt_sb[:, q0 + n0 : q0 + n0 + w],
                                start=True,
                                stop=True,
                            )
                        # E1T = exp(s/8) (causal handled below; dead cols never computed)
                        nc.scalar.activation(
                            e1[:, c, q0:S], sps[:, 0:lw], AF.Exp, scale=0.125
                        )
                        # strict causal mask on the diagonal 128x128 block
                        nc.vector.tensor_mul(
                            e1[:, c, q0 : q0 + 128],
                            e1[:, c, q0 : q0 + 128],
                            dm_sb,
                        )
                        # r1 accumulation: r1[q] += sum_k E1T[k, q]
                        # (split at 512-col PSUM bank boundaries)
                        p = q0
                        while p < S:
                            end = min(S, (p // 512 + 1) * 512)
                            nc.tensor.matmul(
                                r1ps[:, p:end],
                                lhsT=ones_sb,
                                rhs=e1[:, c, p:end],
                                start=(c == 0),
                                stop=(c == NCH - 1),
                                skip_group_check=True,
                            )
                            p = end

                    # ---- phase B: rec1 = 1/r1 (broadcast over partitions) ----
                    rec1 = rcp.tile([128, S], F32, tag="rec1")
                    nc.vector.reciprocal_approx_fast(out=rec1, in_=r1ps[:, 0:S])
                    nc.vector.memset(rec1[:, 0:1], 0.0)

                    # ---- phase C: E2T = exp(E1T * (1-cm)_k * rec1) ----
                    e2 = e2p.tile([128, NCH, S], F32, tag="e2")
                    for c in range(NCH):
                        q0 = 128 * c
                        lw = S - q0
                        tmp = tmpp.tile([128, S], F32, tag="tmp")
                        nc.vector.scalar_tensor_tensor(
                            out=tmp[:, 0:lw],
                            in0=e1[:, c, q0:S],
                            scalar=cm_sb[:, bi * NCH + c : bi * NCH + c + 1],
                            in1=rec1[:, q0:S],
                            op0=ALU.mult,
                            op1=ALU.mult,
                        )
                        nc.scalar.activation(e2[:, c, q0:S], tmp[:, 0:lw], AF.Exp)

                    # ---- phase D: PV matmuls + dead-region fixup + evict ----
                    for m in range(NCH):
                        pvps = pv_psp.tile([128, VW], F32, tag="pv")
                        # rank-1 dead-suffix contribution (incl. r2 dead count)
                        nc.tensor.matmul(
                            pvps[:, 0:129],
                            lhsT=ones_sb[0:1, :],
                            rhs=vs_sb[0:1, m, 0:129],
                            start=True,
                            stop=False,
                            skip_group_check=True,
                        )
                        for c in range(m + 1):
                            nc.tensor.matmul(
                                pvps[:, 0:129],
                                lhsT=e2[:, c, 128 * m : 128 * m + 128],
                                rhs=vc_sb[:, c, 0:129],
                                start=False,
                                stop=(c == m),
                                skip_group_check=True,
                            )
                        rc2 = smallp.tile([128, 1], F32, tag="rc2")
                        nc.vector.reciprocal(out=rc2, in_=pvps[:, 128:129])
                        ot = outp.tile([128, 128], F32, tag="ot")
                        nc.vector.tensor_scalar_mul(ot, pvps[:, 0:128], rc2)
                        if m == 0:
                            nc.vector.memset(ot[0:1, :], 0.0)
                        nc.sync.dma_start(
                            out=out1_d[bi, 128 * m : 128 * m + 128, DK * h : DK * (h + 1)],
                            in_=ot[:, 0:DK],
                        )
                        nc.sync.dma_start(
                            out=out2_d[bi, 128 * m : 128 * m + 128, DK * h : DK * (h + 1)],
                            in_=ot[:, DK : 2 * DK],
                        )

    nc.compile()
    return nc


_NC_CACHE = None


def _get_nc():
    global _NC_CACHE
    if _NC_CACHE is None:
        _NC_CACHE = build_nc()
    return _NC_CACHE


def make_in_maps(q, k, v1, v2, cm):
    """Full inputs -> per-core input maps (host-side sharding + layout)."""
    q = np.asarray(q, dtype=np.float32)
    k = np.asarray(k, dtype=np.float32)
    v1 = np.asarray(v1, dtype=np.float32)
    v2 = np.asarray(v2, dtype=np.float32)
    cm = np.asarray(cm)

    dmask = (np.arange(128)[:, None] < np.arange(128)[None, :]).astype(np.float32)
    onesd = np.ones((128, 128), np.float32)

    in_maps = []
    for core in range(NCORES):
        b0 = core * BLOC
        qs = q[b0 : b0 + BLOC]
        ks = k[b0 : b0 + BLOC]
        qt = np.ascontiguousarray(
            qs.reshape(BLOC, S, H, DK).transpose(0, 2, 3, 1)
        )  # [b, h, dk, s]
        kt = np.ascontiguousarray(ks.reshape(BLOC, S, H, DK).transpose(0, 2, 3, 1))
        v1s = v1[b0 : b0 + BLOC].reshape(BLOC, NCH, 128, H, DK).transpose(0, 3, 1, 2, 4)
        v2s = v2[b0 : b0 + BLOC].reshape(BLOC, NCH, 128, H, DK).transpose(0, 3, 1, 2, 4)
        vcat = np.zeros((BLOC, H, NCH, 128, VW), np.float32)
        vcat[..., 0:DK] = v1s
        vcat[..., DK : 2 * DK] = v2s
        vcat[..., 128] = 1.0
        csum = vcat.sum(axis=3)  # [b, h, c, VW]
        suf = np.flip(np.cumsum(np.flip(csum, axis=2), axis=2), axis=2)
        vsuf = np.ascontiguousarray((suf - csum).astype(np.float32))
        cms = np.ascontiguousarray(
            1.0 - cm[b0 : b0 + BLOC].reshape(BLOC, NCH, 128).astype(np.float32)
        )
        in_maps.append(
            dict(qt=qt, kt=kt, vcat=vcat, vsuf=vsuf, cms=cms, dmask=dmask, onesd=onesd)
        )
    return in_maps


def kernel(q, k, v1, v2, counter_attention_mask):
    global LAST_RESULTS
    in_maps = make_in_maps(q, k, v1, v2, counter_attention_mask)
    nc = _get_nc()
    res = run_bass_kernel_spmd(
        nc, in_maps, core_ids=list(range(NCORES)), trace=TRACE
    )
    LAST_RESULTS = res
    out1 = np.concatenate([r["out1"] for r in res.results], axis=0)
    out2 = np.concatenate([r["out2"] for r in res.results], axis=0)
    return out1, out2
